# revision 9
# baseline (speedup 1.0000x reference)
"""Decode-step multi-head attention (KV cache) on 8 Trainium2 NeuronCores.

Full inputs in, full outputs out.  Tensor-parallel over heads: each of the
8 cores owns 4 of the 32 heads (wq/wk/wv column shards, wo row shard, head
slice of both KV caches).  The all-reduce after the output projection and
the head-axis gather of the attention weights happen on the host.

Problem shape (hardcoded):
  x[16,1,4096], wq/wk/wv/wo[4096,4096], cache_k/v[16,4096,32,128],
  mask[1,1,1,4096], start_pos=4095, H=32 heads, dqk=dv=128.
Returns (weight[16,32,1,4096], output[16,1,4096]) like the reference.

Per-core dataflow (all fp32):
  - q^T/k_new^T/v_new^T via weight-stationary projections (outputs land
    pre-transposed as [d, pair] columns; pair = local_head*16 + batch).
  - scores: per pair, 32 K-tile-stationary matmuls -> PSUM [s%128, s//128];
    the KV-cache append is a 1-column SBUF patch of the K slab.
  - softmax without max-subtraction (scores here are bounded ~|8|):
    exp on ACT with accumulated row sums, partition-reduce via ones-matmul,
    reciprocal broadcast back via a K=1 matmul.
  - the stale cache_v row at s=4095 is neutralized by zeroing the e entries
    for s=4095 (after extracting them with a selector matmul) and adding
    the rank-1 correction xv^T * e_last to the context instead.
  - ctx: per pair, 32 V-tile-stationary accumulating matmuls.
  - output projection with ctx^T blocks stationary.
DMA discipline: big loads (K/V slabs, weights) ride the SP HWDGE ring
alone and fully linear; stores ride GPSIMD SWDGE so no compute-dependent
DMA ever blocks the load ring.
"""

import numpy as np

BS = 16
DIM = 4096
H = 32
DQK = 128
DV = 128
MAX_SEQ = 4096
START_POS = 4095
NCORES = 8
HL = H // NCORES          # 4 local heads per core
NPAIR = HL * BS           # 64 (head, batch) pairs per core; pair = h*16 + b
NTILE = MAX_SEQ // 128    # 32 seq tiles of 128
SCALE = DQK ** -0.5
WGRP = 8                  # pairs per weight-output DMA batch

_CACHE = {}


def _build_nc():
    """Build the per-core Bass program (same program on all 8 cores)."""
    from contextlib import ExitStack

    import concourse.mybir as mybir
    import concourse.tile as tile
    from concourse import bacc

    f32 = mybir.dt.float32
    # Bacc (not plain Bass): finalize() -> compile() runs the wait-splitting
    # passes without which walrus rejects multi-wait instructions.
    nc = bacc.Bacc()

    # ---- DRAM parameters (per-core shards, host-prepared layouts) ----
    # xT[p, k*16+b] = x[b, k*128+p]
    xT_d = nc.declare_dram_parameter("xT", [128, NTILE * BS], f32, isOutput=False)
    # maskT[p, j] = mask[j*128+p]
    maskT_d = nc.declare_dram_parameter("maskT", [128, NTILE], f32, isOutput=False)
    # w[h][dim][c]: head-major column shards (contiguous per head)
    wq_d = nc.declare_dram_parameter("wq", [HL, DIM, DQK], f32, isOutput=False)
    wk_d = nc.declare_dram_parameter("wk", [HL, DIM, DQK], f32, isOutput=False)
    wv_d = nc.declare_dram_parameter("wv", [HL, DIM, DV], f32, isOutput=False)
    # wo[c][h][dv][chunk]: chunk-major row shard tiles
    wo_d = nc.declare_dram_parameter("wo", [8, HL, DV, 512], f32, isOutput=False)
    # kT[pair][d][s] = cache_k[b, s, head, d]
    kT_d = nc.declare_dram_parameter("kT", [NPAIR, DQK, MAX_SEQ], f32, isOutput=False)
    # v[pair][q][j*128+d] = cache_v[b, j*128+q, head, d]  (s = j*128+q)
    v_d = nc.declare_dram_parameter("v", [NPAIR, 128, NTILE * DV], f32, isOutput=False)
    # sel[p] = 1 at p=127 else 0; selmask = 1 - sel  (host-provided constants;
    # engine ops may not start at partition 127, so these can't be memset)
    sel_d = nc.declare_dram_parameter("sel", [128, 1], f32, isOutput=False)
    selmask_d = nc.declare_dram_parameter("selmask", [128, 1], f32, isOutput=False)
    # weight_out[pair, p, j] = scores(s = j*128+p) + mask
    weight_d = nc.declare_dram_parameter(
        "weight", [NPAIR, 128, NTILE], f32, isOutput=True
    )
    out_d = nc.declare_dram_parameter("out", [BS, DIM], f32, isOutput=True)

    Exp = mybir.ActivationFunctionType.Exp

    with tile.TileContext(nc) as tc, ExitStack() as ctx:
        singles = ctx.enter_context(tc.tile_pool(name="singles", bufs=1))
        wpool = ctx.enter_context(tc.tile_pool(name="wpool", bufs=2))
        kpool = ctx.enter_context(tc.tile_pool(name="kpool", bufs=4))
        vpool = ctx.enter_context(tc.tile_pool(name="vpool", bufs=4))
        spool = ctx.enter_context(tc.tile_pool(name="spool", bufs=3))
        ppool = ctx.enter_context(tc.tile_pool(name="ppool", bufs=2, space="PSUM"))
        scorep = ctx.enter_context(tc.tile_pool(name="scorep", bufs=4, space="PSUM"))
        ctxp = ctx.enter_context(tc.tile_pool(name="ctxp", bufs=1, space="PSUM"))
        redp = ctx.enter_context(tc.tile_pool(name="redp", bufs=1, space="PSUM"))

        # ---- persistent SBUF ----
        xT_sb = singles.tile([128, NTILE * BS], f32, tag="xT")
        nc.sync.dma_start(out=xT_sb, in_=xT_d[:])
        maskT_sb = singles.tile([128, NTILE], f32, tag="maskT")
        nc.sync.dma_start(out=maskT_sb, in_=maskT_d[:])

        qT_sb = singles.tile([128, NPAIR], f32, tag="qT")       # q^T, pre-scaled
        kTn_sb = singles.tile([128, NPAIR], f32, tag="kTn")     # new key^T
        xvT_sb = singles.tile([128, NPAIR], f32, tag="xvT")     # new value^T
        e_sb = singles.tile([128, NPAIR * NTILE], f32, tag="e") # exp(scores)
        rowsum_sb = singles.tile([128, NPAIR], f32, tag="rowsum")
        ones_col = singles.tile([128, 1], f32, tag="ones_col")
        nc.vector.memset(ones_col, 1.0)
        ones_row = singles.tile([1, 128], f32, tag="ones_row")
        nc.vector.memset(ones_row, 1.0)
        sel_col = singles.tile([128, 1], f32, tag="sel_col")    # e_127 selector
        nc.sync.dma_start(out=sel_col, in_=sel_d[:])
        selmask_col = singles.tile([128, 1], f32, tag="selmask_col")
        nc.sync.dma_start(out=selmask_col, in_=selmask_d[:])
        recip_sb = singles.tile([1, NPAIR], f32, tag="recip")
        recipb_sb = singles.tile([128, NPAIR], f32, tag="recipb")
        elast_sb = singles.tile([1, NPAIR], f32, tag="elast")
        elastb_sb = singles.tile([128, NPAIR], f32, tag="elastb")
        corr_sb = singles.tile([128, NPAIR], f32, tag="corr")
        ctx_sb = singles.tile([128, NPAIR], f32, tag="ctxsb")
        ctx2_sb = singles.tile([128, NPAIR], f32, tag="ctx2sb")
        out_sb = singles.tile([BS, DIM], f32, tag="outsb")

        # ---- projections: q^T (scaled), k_new^T, v_new^T ----
        for wdram, dest, scl in (
            (wq_d, qT_sb, SCALE),
            (wk_d, kTn_sb, 1.0),
            (wv_d, xvT_sb, 1.0),
        ):
            for h in range(HL):
                wh = wpool.tile([128, NTILE, 128], f32, tag="wpool")
                nc.sync.dma_start(
                    out=wh, in_=wdram[h].rearrange("(k p) c -> p k c", p=128)
                )
                pq = ppool.tile([128, BS], f32, tag="ppool")
                for k in range(NTILE):
                    nc.tensor.matmul(
                        pq,
                        lhsT=wh[:, k, :],
                        rhs=xT_sb[:, k * BS : (k + 1) * BS],
                        start=(k == 0),
                        stop=(k == NTILE - 1),
                    )
                if scl != 1.0:
                    nc.scalar.mul(dest[:, h * BS : (h + 1) * BS], pq, scl)
                else:
                    nc.vector.tensor_copy(dest[:, h * BS : (h + 1) * BS], pq)

        # ---- scores sweep (grouped for batched weight writeback) ----
        for g in range(NPAIR // WGRP):
            sgroup = spool.tile([128, WGRP * NTILE], f32, tag="spool")
            for pg in range(WGRP):
                pair = g * WGRP + pg
                ks = kpool.tile([128, MAX_SEQ], f32, tag="kpool")
                nc.sync.dma_start(out=ks, in_=kT_d[pair])
                # cache append: overwrite column s=4095 with the new key
                nc.vector.tensor_copy(ks[:, 4095:4096], kTn_sb[:, pair : pair + 1])
                ps = scorep.tile([128, NTILE], f32, tag="scorep")
                for j in range(NTILE):
                    nc.tensor.matmul(
                        ps[:, j : j + 1],
                        lhsT=ks[:, j * 128 : (j + 1) * 128],
                        rhs=qT_sb[:, pair : pair + 1],
                        start=True,
                        stop=True,
                    )
                ssb = sgroup[:, pg * NTILE : (pg + 1) * NTILE]
                nc.vector.tensor_add(ssb, ps, maskT_sb)
                nc.scalar.activation(
                    e_sb[:, pair * NTILE : (pair + 1) * NTILE],
                    ssb,
                    Exp,
                    accum_out=rowsum_sb[:, pair : pair + 1],
                )
            # SBUF APs must keep the partition dim first; reorder the DRAM side
            nc.gpsimd.dma_start(
                out=weight_d[g * WGRP : (g + 1) * WGRP].rearrange("g p j -> p g j"),
                in_=sgroup.rearrange("p (g j) -> p g j", j=NTILE),
            )

        # ---- softmax denominators + last-position extraction ----
        e3 = e_sb.rearrange("p (g j) -> p g j", j=NTILE)
        pd = redp.tile([1, NPAIR], f32, tag="redp")
        nc.tensor.matmul(pd, lhsT=ones_col, rhs=rowsum_sb, start=True, stop=True)
        nc.vector.reciprocal(recip_sb, pd)
        pb = redp.tile([128, NPAIR], f32, tag="redp")
        nc.tensor.matmul(pb, lhsT=ones_row, rhs=recip_sb, start=True, stop=True)
        nc.vector.tensor_copy(recipb_sb, pb)
        # e_last[pair] = e[127, pair*32+31]  (the s=4095 attention weight)
        pe_last = redp.tile([1, NPAIR], f32, tag="redp")
        nc.tensor.matmul(pe_last, lhsT=sel_col, rhs=e3[:, :, 31], start=True, stop=True)
        nc.vector.tensor_copy(elast_sb, pe_last)
        pb2 = redp.tile([128, NPAIR], f32, tag="redp")
        nc.tensor.matmul(pb2, lhsT=ones_row, rhs=elast_sb, start=True, stop=True)
        nc.vector.tensor_copy(elastb_sb, pb2)
        # zero the s=4095 e entries so the stale cached V row contributes 0
        nc.vector.tensor_scalar_mul(e3[:, :, 31], e3[:, :, 31], selmask_col)

        # ---- ctx sweep: V slabs stationary, accumulate per pair column ----
        pc = ctxp.tile([128, NPAIR], f32, tag="ctxp")
        for pair in range(NPAIR):
            vs = vpool.tile([128, NTILE * 128], f32, tag="vpool")
            nc.sync.dma_start(out=vs, in_=v_d[pair])
            for j in range(NTILE):
                nc.tensor.matmul(
                    pc[:, pair : pair + 1],
                    lhsT=vs[:, j * 128 : (j + 1) * 128],
                    rhs=e_sb[:, pair * NTILE + j : pair * NTILE + j + 1],
                    start=(j == 0),
                    stop=(j == NTILE - 1),
                )
        # ctx = (ctx_unnorm + xv^T * e_last) / denom
        nc.vector.tensor_mul(corr_sb, xvT_sb, elastb_sb)
        nc.vector.tensor_add(ctx_sb, pc, corr_sb)
        nc.vector.tensor_mul(ctx2_sb, ctx_sb, recipb_sb)

        # ---- output projection: out[b, dim] = sum_h ctx[:, h] @ wo rows ----
        for c in range(8):
            po = ppool.tile([BS, 512], f32, tag="ppool")
            for h in range(HL):
                wo_t = wpool.tile([128, 512], f32, tag="wpool")
                nc.sync.dma_start(out=wo_t, in_=wo_d[c, h])
                nc.tensor.matmul(
                    po,
                    lhsT=ctx2_sb[:, h * BS : (h + 1) * BS],
                    rhs=wo_t,
                    start=(h == 0),
                    stop=(h == HL - 1),
                )
            nc.scalar.copy(out_sb[:, c * 512 : (c + 1) * 512], po)
        nc.gpsimd.dma_start(out=out_d[:], in_=out_sb)

    nc.finalize()
    return nc


def _get_nc():
    if "nc" not in _CACHE:
        _CACHE["nc"] = _build_nc()
    return _CACHE["nc"]


def _make_in_maps(inputs):
    x = np.ascontiguousarray(np.asarray(inputs["x"], dtype=np.float32))
    mask = np.ascontiguousarray(np.asarray(inputs["mask"], dtype=np.float32))
    wq = np.asarray(inputs["wq"], dtype=np.float32)
    wk = np.asarray(inputs["wk"], dtype=np.float32)
    wv = np.asarray(inputs["wv"], dtype=np.float32)
    wo = np.asarray(inputs["wo"], dtype=np.float32)
    cache_k = np.asarray(inputs["cache_k"], dtype=np.float32)
    cache_v = np.asarray(inputs["cache_v"], dtype=np.float32)

    # x[b, 0, dim] -> xT[p, k*16+b] with dim = k*128+p
    xT = np.ascontiguousarray(
        x.reshape(BS, NTILE, 128).transpose(2, 1, 0).reshape(128, NTILE * BS)
    )
    # mask[..., s] -> maskT[p, j] with s = j*128+p
    maskT = np.ascontiguousarray(mask.reshape(NTILE, 128).T)

    in_maps = []
    for c in range(NCORES):
        cols = slice(c * HL * DQK, (c + 1) * HL * DQK)
        # head-major weight shards: [4, 4096, 128]
        wq_c = np.ascontiguousarray(
            wq[:, cols].reshape(DIM, HL, DQK).transpose(1, 0, 2)
        )
        wk_c = np.ascontiguousarray(
            wk[:, cols].reshape(DIM, HL, DQK).transpose(1, 0, 2)
        )
        wv_c = np.ascontiguousarray(
            wv[:, cols].reshape(DIM, HL, DV).transpose(1, 0, 2)
        )
        # chunk-major wo tiles: [8, 4, 128, 512]
        wo_c = np.ascontiguousarray(
            wo[cols, :].reshape(HL, DV, 8, 512).transpose(2, 0, 1, 3)
        )
        ck = cache_k[:, :, c * HL : (c + 1) * HL, :]  # [16, 4096, 4, 128]
        cv = cache_v[:, :, c * HL : (c + 1) * HL, :]
        kT_c = np.ascontiguousarray(ck.transpose(2, 0, 3, 1)).reshape(
            NPAIR, DQK, MAX_SEQ
        )
        # v[pair][q][j*128+d]: fully linear slab reads
        v_c = np.ascontiguousarray(
            np.ascontiguousarray(cv.transpose(2, 0, 1, 3))
            .reshape(NPAIR, NTILE, 128, DV)
            .transpose(0, 2, 1, 3)
        ).reshape(NPAIR, 128, NTILE * DV)
        sel = np.zeros((128, 1), np.float32)
        sel[127, 0] = 1.0
        selmask = np.ones((128, 1), np.float32)
        selmask[127, 0] = 0.0
        in_maps.append(
            {
                "xT": xT,
                "maskT": maskT,
                "sel": sel,
                "selmask": selmask,
                "wq": wq_c,
                "wk": wk_c,
                "wv": wv_c,
                "wo": wo_c,
                "kT": kT_c,
                "v": v_c,
            }
        )
    return in_maps


def _run(inputs, trace=False, trace_cores=None):
    from concourse.bass_utils import run_bass_kernel_spmd

    nc = _get_nc()
    in_maps = _make_in_maps(inputs)
    res = run_bass_kernel_spmd(
        nc,
        in_maps,
        list(range(NCORES)),
        trace=trace,
        trace_cores=trace_cores,
    )

    weights = []
    out = np.zeros((BS, DIM), dtype=np.float32)
    for c in range(NCORES):
        r = res.results[c]
        w = np.asarray(r["weight"])  # [64, 128, 32]
        w = (
            w.reshape(HL, BS, 128, NTILE)
            .transpose(1, 0, 3, 2)
            .reshape(BS, HL, MAX_SEQ)
        )
        weights.append(w)
        out += np.asarray(r["out"])
    weight = np.concatenate(weights, axis=1)[:, :, None, :]  # [16, 32, 1, 4096]
    output = out[:, None, :]  # [16, 1, 4096]
    return (weight.astype(np.float32), output.astype(np.float32)), res


def kernel(**inputs):
    (weight, output), _ = _run(inputs, trace=False)
    return weight, output
